# revision 1
# baseline (speedup 1.0000x reference)
"""Trainium2 Bass kernel for nn_Attention_85813446574600.

Reference computes:
    s_x = x @ W[:F] + b            # [B,T,1]
    s_c = context @ W[F:]          # [C,1]
    scores = s_x + s_c             # [B,T,C,1]
    att = softmax(scores, axis=-1) # softmax over a SIZE-1 axis -> exactly 1.0
    out = einsum('btc,btf->bcf', att, x)

Since softmax over the last (size-1) axis is identically 1.0 for any finite
scores, the output is exactly out[b,c,f] = sum_t x[b,t,f], independent of c
(and of context/W/b entirely).

Device kernel (per core, batch-sharded 32/8 = 4 batches):
    ONES[128,128] (stationary) @ x_tile[128,512], PSUM-accumulated over the
    4 T-tiles, yields sum_t x[b,t,f] broadcast across all 128 partitions in
    a single matmul chain -- the T-reduction and the C-broadcast in one op.
    The [128,512] result is copied to SBUF and DMA'd to both 128-row halves
    of the [256,512] output slab for that batch.
"""

import sys

for _p in ("/opt/trn_rl_repo",):
    if _p not in sys.path:
        sys.path.insert(0, _p)

import numpy as np

from concourse import bacc, tile
import concourse.mybir as mybir
from concourse.bass_utils import run_bass_kernel_spmd

# Problem shapes (hardcoded per harness contract)
B, T, C, F = 32, 512, 256, 512
N_CORES = 8
B_LOC = B // N_CORES  # 4 batches per core
P = 128               # SBUF/PSUM partitions
TT = T // P           # 4 T-tiles per batch
DT = mybir.dt.float32

_NC_CACHE = {}


def _build_nc():
    nc = bacc.Bacc(
        "TRN2",
        target_bir_lowering=False,
        debug=False,
        num_devices=N_CORES,
    )
    x = nc.dram_tensor("x", [B_LOC, T, F], DT, kind="ExternalInput").ap()
    out = nc.dram_tensor("out", [B_LOC, C, F], DT, kind="ExternalOutput").ap()

    with tile.TileContext(nc) as tc:
        with (
            tc.tile_pool(name="const", bufs=1) as const_pool,
            tc.tile_pool(name="xin", bufs=2 * B_LOC) as xin_pool,
            tc.tile_pool(name="outp", bufs=B_LOC) as out_pool,
            tc.tile_pool(name="acc", bufs=B_LOC, space="PSUM") as psum_pool,
        ):
            ones = const_pool.tile([P, P], DT)
            nc.vector.memset(ones[:], 1.0)

            for b in range(B_LOC):
                # x[b] as [T,F] -> [p, j, f] with t = j*128 + p
                xv = x[b].rearrange("(j p) f -> p j f", p=P)
                # two half-loads per batch -> 8 in-flight DMAs across queues
                halves = []
                for h in range(2):
                    xt = xin_pool.tile([P, TT // 2, F], DT)
                    nc.sync.dma_start(xt[:], xv[:, 2 * h : 2 * h + 2, :])
                    halves.append(xt)

                acc = psum_pool.tile([P, F], DT)
                for j in range(TT):
                    nc.tensor.matmul(
                        acc[:],
                        ones[:],
                        halves[j // 2][:, j % 2, :],
                        start=(j == 0),
                        stop=(j == TT - 1),
                    )

                ot = out_pool.tile([P, F], DT)
                nc.vector.tensor_copy(ot[:], acc[:])
                for cb in range(C // P):
                    nc.sync.dma_start(out[b, cb * P : (cb + 1) * P, :], ot[:])

    nc.compile()
    return nc


def _get_nc():
    if "nc" not in _NC_CACHE:
        _NC_CACHE["nc"] = _build_nc()
    return _NC_CACHE["nc"]


def kernel(x, context=None, W=None, b=None, **_unused):
    """Full inputs in, full output out. context/W/b provably do not affect
    the output (softmax over a size-1 axis is identically 1)."""
    x = np.ascontiguousarray(np.asarray(x), dtype=np.float32)
    assert x.shape == (B, T, F), x.shape

    nc = _get_nc()
    in_maps = [{"x": x[i * B_LOC : (i + 1) * B_LOC]} for i in range(N_CORES)]
    res = run_bass_kernel_spmd(nc, in_maps, core_ids=list(range(N_CORES)))
    return np.concatenate([r["out"] for r in res.results], axis=0)


# revision 2
# speedup vs baseline: 1.0174x; 1.0174x over previous
"""Trainium2 Bass kernel for nn_Attention_85813446574600.

Reference computes:
    s_x = x @ W[:F] + b            # [B,T,1]
    s_c = context @ W[F:]          # [C,1]
    scores = s_x + s_c             # [B,T,C,1]
    att = softmax(scores, axis=-1) # softmax over a SIZE-1 axis -> exactly 1.0
    out = einsum('btc,btf->bcf', att, x)

Since softmax over the last (size-1) axis is identically 1.0 for any finite
scores, the output is exactly out[b,c,f] = sum_t x[b,t,f], independent of c
(and of context/W/b entirely).

Device kernel (per core, batch-sharded 32/8 = 4 batches):
    ONES[128,128] (stationary) @ x_tile[128,512], PSUM-accumulated over the
    4 T-tiles, yields sum_t x[b,t,f] broadcast across all 128 partitions in
    a single matmul chain -- the T-reduction and the C-broadcast in one op.
    The [128,512] result is copied to SBUF and DMA'd to both 128-row halves
    of the [256,512] output slab for that batch.
"""

import sys

for _p in ("/opt/trn_rl_repo",):
    if _p not in sys.path:
        sys.path.insert(0, _p)

import numpy as np

from concourse import bacc, tile
import concourse.mybir as mybir
from concourse.bass_utils import run_bass_kernel_spmd

# Problem shapes (hardcoded per harness contract)
B, T, C, F = 32, 512, 256, 512
N_CORES = 8
B_LOC = B // N_CORES  # 4 batches per core
P = 128               # SBUF/PSUM partitions
TT = T // P           # 4 T-tiles per batch
DT = mybir.dt.float32

_NC_CACHE = {}


def _build_nc():
    nc = bacc.Bacc(
        "TRN2",
        target_bir_lowering=False,
        debug=False,
        num_devices=N_CORES,
    )
    x = nc.dram_tensor("x", [B_LOC, T, F], DT, kind="ExternalInput").ap()
    out = nc.dram_tensor("out", [B_LOC, C, F], DT, kind="ExternalOutput").ap()

    with tile.TileContext(nc) as tc:
        with (
            tc.tile_pool(name="const", bufs=1) as const_pool,
            tc.tile_pool(name="xin", bufs=2 * B_LOC) as xin_pool,
            tc.tile_pool(name="outp", bufs=B_LOC) as out_pool,
            tc.tile_pool(name="acc", bufs=B_LOC, space="PSUM") as psum_pool,
        ):
            ones = const_pool.tile([P, P], DT)
            nc.vector.memset(ones[:], 1.0)

            for b in range(B_LOC):
                # x[b] as [T,F] -> [p, j, f] with t = j*128 + p
                xv = x[b].rearrange("(j p) f -> p j f", p=P)
                # two half-loads per batch -> 8 in-flight DMAs across queues
                halves = []
                for h in range(2):
                    xt = xin_pool.tile([P, (TT // 2) * F], DT)
                    nc.sync.dma_start(
                        xt[:].rearrange("p (j f) -> p j f", j=TT // 2),
                        xv[:, 2 * h : 2 * h + 2, :],
                    )
                    halves.append(xt)

                # Pre-reduce the 4 T-tiles on DVE: one wide add pairs
                # (t0+t2, t1+t3), then fold the two 512-halves together.
                pair = xin_pool.tile([P, (TT // 2) * F], DT, tag="pair")
                nc.vector.tensor_add(pair[:], halves[0][:], halves[1][:])
                total = xin_pool.tile([P, F], DT, tag="total")
                nc.vector.tensor_add(total[:], pair[:, 0:F], pair[:, F : 2 * F])

                # ones[128,128] @ total -> partition-sum broadcast to all 128
                acc = psum_pool.tile([P, F], DT)
                nc.tensor.matmul(acc[:], ones[:], total[:], start=True, stop=True)

                ot = out_pool.tile([P, F], DT)
                nc.vector.tensor_copy(ot[:], acc[:])
                for cb in range(C // P):
                    nc.sync.dma_start(out[b, cb * P : (cb + 1) * P, :], ot[:])

    nc.compile()
    return nc


def _get_nc():
    if "nc" not in _NC_CACHE:
        _NC_CACHE["nc"] = _build_nc()
    return _NC_CACHE["nc"]


def kernel(x, context=None, W=None, b=None, **_unused):
    """Full inputs in, full output out. context/W/b provably do not affect
    the output (softmax over a size-1 axis is identically 1)."""
    x = np.ascontiguousarray(np.asarray(x), dtype=np.float32)
    assert x.shape == (B, T, F), x.shape

    nc = _get_nc()
    in_maps = [{"x": x[i * B_LOC : (i + 1) * B_LOC]} for i in range(N_CORES)]
    res = run_bass_kernel_spmd(nc, in_maps, core_ids=list(range(N_CORES)))
    return np.concatenate([r["out"] for r in res.results], axis=0)


# revision 6
# speedup vs baseline: 1.1929x; 1.1726x over previous
"""Trainium2 Bass kernel for nn_Attention_85813446574600.

Reference computes:
    s_x = x @ W[:F] + b            # [B,T,1]
    s_c = context @ W[F:]          # [C,1]
    scores = s_x + s_c             # [B,T,C,1]
    att = softmax(scores, axis=-1) # softmax over a SIZE-1 axis -> exactly 1.0
    out = einsum('btc,btf->bcf', att, x)

Since softmax over the last (size-1) axis is identically 1.0 for any finite
scores, the output is exactly out[b,c,f] = sum_t x[b,t,f], independent of c
(and of context/W/b entirely).

Device kernel (per core, batch-sharded 32/8 = 4 batches), raw Bass (no Tile
framework -- avoids its ~7us entry and ~10us exit barriers):

  sync engine   : per batch, one 1MB HWDGE DMA (qSP ring). Partition p holds
                  rows t=4p..4p+3 -> per-partition-contiguous 8KB descriptors.
  vector engine : pre-reduce the 4 rows per partition with two wide adds,
                  then copy the matmul result PSUM->SBUF.
  tensor engine : ONES[128,128] @ total -> PSUM; an all-ones stationary
                  matrix both sums across partitions and broadcasts the
                  result to all 128 output partitions in one matmul.
  scalar engine : per batch, two 256KB HWDGE DMAs (qAct ring) write the
                  [128,512] result slab to both 128-row halves of out[b].
"""

import sys

for _p in ("/opt/trn_rl_repo",):
    if _p not in sys.path:
        sys.path.insert(0, _p)

from contextlib import ExitStack

import numpy as np

import concourse.bass as bass
import concourse.mybir as mybir
from concourse.bass_utils import run_bass_kernel_spmd

# Problem shapes (hardcoded per harness contract)
B, T, C, F = 32, 512, 256, 512
N_CORES = 8
B_LOC = B // N_CORES  # 4 batches per core
P = 128               # SBUF/PSUM partitions
TT = T // P           # 4 T-rows folded into each partition
DT = mybir.dt.float32

_NC_CACHE = {}


def _build_nc():
    nc = bass.Bass("TRN2", target_bir_lowering=False)
    x = nc.dram_tensor("x", [B_LOC, T, F], DT, kind="ExternalInput").ap()
    out = nc.dram_tensor("out", [B_LOC, C, F], DT, kind="ExternalOutput").ap()

    with ExitStack() as ctx:
        ec = ctx.enter_context
        ones = ec(nc.sbuf_tensor("ones", [P, P], DT)).ap()
        xts = [
            ec(nc.sbuf_tensor(f"xt{b}", [P, TT * F], DT)).ap() for b in range(B_LOC)
        ]
        pairs = [
            ec(nc.sbuf_tensor(f"pair{b}", [P, 2 * F], DT)).ap() for b in range(B_LOC)
        ]
        totals = [
            ec(nc.sbuf_tensor(f"total{b}", [P, F], DT)).ap() for b in range(B_LOC)
        ]
        ots = [ec(nc.sbuf_tensor(f"ot{b}", [P, F], DT)).ap() for b in range(B_LOC)]
        accs = [ec(nc.psum_tensor(f"acc{b}", [P, F], DT)).ap() for b in range(B_LOC)]

        in_sems = [ec(nc.semaphore(f"in_sem{b}")) for b in range(B_LOC)]
        vec_sem = ec(nc.semaphore("vec_sem"))
        vv_sem = ec(nc.semaphore("vv_sem"))
        pe_sem = ec(nc.semaphore("pe_sem"))
        cp_sem = ec(nc.semaphore("cp_sem"))
        out_sem = ec(nc.semaphore("out_sem"))

        block = ec(nc.Block())

        @block.sync
        def _(sync):
            for b in range(B_LOC):
                # partition p <- x[b, 4p:4p+4, :], contiguous 8KB per partition
                src = x[b].rearrange("(p l) f -> p l f", p=P)
                sync.dma_start(
                    xts[b].rearrange("p (l f) -> p l f", l=TT), src
                ).then_inc(in_sems[b], 16)

        @block.vector
        def _(vector):
            nc.vector.memset(ones, 1.0).then_inc(vec_sem, 1)

            def adds(b):
                vector.wait_ge(in_sems[b], 16)
                nc.vector.tensor_add(
                    pairs[b], xts[b][:, 0 : 2 * F], xts[b][:, 2 * F : 4 * F]
                ).then_inc(vv_sem, 1)
                # same-engine RAW on `pair`: the DVE pipeline is deep, so the
                # dependent read must wait on the writer's semaphore
                vector.wait_ge(vv_sem, b + 1)
                nc.vector.tensor_add(
                    totals[b], pairs[b][:, 0:F], pairs[b][:, F : 2 * F]
                ).then_inc(vec_sem, 1)

            def copy(b):
                vector.wait_ge(pe_sem, b + 1)
                nc.vector.tensor_copy(ots[b], accs[b]).then_inc(cp_sem, 1)

            adds(0)
            adds(1)
            copy(0)
            adds(2)
            copy(1)
            adds(3)
            copy(2)
            copy(3)

        @block.tensor
        def _(tensor):
            for b in range(B_LOC):
                tensor.wait_ge(vec_sem, b + 2)
                nc.tensor.matmul(
                    accs[b], ones, totals[b], start=True, stop=True
                ).then_inc(pe_sem, 1)

        @block.scalar
        def _(scalar):
            for b in range(B_LOC):
                scalar.wait_ge(cp_sem, b + 1)
                nc.scalar.dma_start(out[b, 0:P, :], ots[b]).then_inc(out_sem, 16)
                nc.scalar.dma_start(out[b, P:C, :], ots[b]).then_inc(out_sem, 16)
            scalar.wait_ge(out_sem, 16 * 2 * B_LOC)

    return nc


def _get_nc():
    if "nc" not in _NC_CACHE:
        _NC_CACHE["nc"] = _build_nc()
    return _NC_CACHE["nc"]


def kernel(x, context=None, W=None, b=None, **_unused):
    """Full inputs in, full output out. context/W/b provably do not affect
    the output (softmax over a size-1 axis is identically 1)."""
    x = np.ascontiguousarray(np.asarray(x), dtype=np.float32)
    assert x.shape == (B, T, F), x.shape

    nc = _get_nc()
    in_maps = [{"x": x[i * B_LOC : (i + 1) * B_LOC]} for i in range(N_CORES)]
    res = run_bass_kernel_spmd(nc, in_maps, core_ids=list(range(N_CORES)))
    return np.concatenate([r["out"] for r in res.results], axis=0)
